# revision 12
# baseline (speedup 1.0000x reference)
"""Trainium2 Bass kernel for nn_BICEPNeuralLayer.

Math: the reference module (Euler-Maruyama SDE scan -> Conv1d over time ->
time-mean -> linear projection) is LINEAR in the noise tensor, so the whole
pipeline collapses algebraically:

  paths[t] = c_b * sum_s retain^(t-s) eps_s          (c_b = feedback_b*sqrt(dt))
  mean_t(conv(paths)) folds to per-timestep weights on eps:
     out[b] = (c_b/NS) * (Tsum @ A[b] - T0 @ L[b] - T2 @ F[b]) + bias
  A[b,i] = sum_s gA[s] noise[b,s,i],   gA[s] = (1-retain^(NS-s))/(1-retain)
  L[b,i] = sum_s retain^(NS-1-s) noise[b,s,i]
  F[b,i] = noise[b,0,i]
  Tsum = out_w @ (W0+W1+W2), T0 = out_w @ W0, T2 = out_w @ W2  (Wk = conv_w[:,:,k])
  bias  = out_w @ conv_b + out_b

Device work per core (pure data parallel over batch, 32 samples/core):
  stage 1: per (sample, feature-chunk): matmul(lhsT=noise_chunk[128s x 128i]
           fp16, rhs=G3[128s x 3] fp16) -> psum[i, {A,L,F}] fp32.
           noise is cast fp32->fp16 on the host before upload, halving the
           HBM read (the roofline term) and avoiding the fp32 double-pass
           weight-load penalty on the PE.
  stage 2: 24 accumulating matmuls lhsT=V[128i x 32b] fp16, rhs=Mcat
           [128i x 512j] fp16 -> psum[32b, 512j] fp32, then scale by
           per-sample c_b (host-precomputed sigmoid, tiny) and add bias.
"""

import sys

if "/opt/trn_rl_repo" not in sys.path:
    sys.path.insert(0, "/opt/trn_rl_repo")

from contextlib import ExitStack

import numpy as np

import concourse.bass as bass
import concourse.tile as tile
from concourse import mybir
from concourse.bass_utils import run_bass_kernel_spmd

B, IN, OUT, P, NS = 256, 1024, 512, 1000, 128
NCORES = 8
BSH = B // NCORES  # 32 samples per core
NG = 16            # noise DMA groups per core
GB = BSH // NG     # samples per DMA group (~0.5 MB fp16 per dma_start)
NQ = 8             # feature chunks: 7*128 + 104 = 1000
LASTM = P - (NQ - 1) * 128  # 104

F32 = mybir.dt.float32
F16 = mybir.dt.float16
F16_NP = mybir.dt.np(F16)

_CACHE = {}

LAST_RUN = None  # BassKernelResults of the most recent execution (for test.py)


def _chunk_m(q: int) -> int:
    return 128 if q < NQ - 1 else LASTM


def _split_sync_waits(nc: bass.Bass, max_waits: int = 1) -> int:
    """Walrus in this container accepts at most one sync-wait command per
    instruction. Tile emits instructions (notably the epilogue Drain and any
    op depending on two DMA queues) with several waits. Split the surplus
    onto single-wait NoOps inserted just before, on the same engine, which
    is semantically identical for sem-ge waits."""
    nid = 0
    for fn in nc.m.functions:
        for bb in fn.blocks:
            insts = list(bb.instructions)
            out, changed = [], False
            for inst in insts:
                si = inst.sync_info
                if si is not None and si.on_wait and len(si.on_wait) > max_waits:
                    waits = list(si.on_wait)
                    extra, keep = waits[:-max_waits], waits[-max_waits:]
                    for w in extra:
                        nid += 1
                        out.append(
                            mybir.InstNoOp(
                                name=f"waitsplit-{nid}",
                                sync_info=mybir.SyncInfo(on_wait=[w], on_update=[]),
                                bass_nofuse=True,
                                engine=inst.engine,
                            )
                        )
                    inst.sync_info = mybir.SyncInfo(
                        on_wait=keep, on_update=list(si.on_update)
                    )
                    changed = True
                out.append(inst)
            if changed:
                bb.instructions = out
    return nid


def _build_program() -> bass.Bass:
    if "nc" in _CACHE:
        return _CACHE["nc"]

    nc = bass.Bass()

    noise_d = nc.dram_tensor("noise_sh", [BSH, NS, P], F16, kind="ExternalInput")
    g3_d = nc.dram_tensor("g3", [NS, 3], F16, kind="ExternalInput")
    mcat_d = nc.dram_tensor("mcat", [128, 3 * NQ, OUT], F16, kind="ExternalInput")
    c_d = nc.dram_tensor("cvec", [1, 3 * BSH], F32, kind="ExternalInput")
    bias_d = nc.dram_tensor("biasv", [1, OUT], F32, kind="ExternalInput")
    out_d = nc.dram_tensor("out", [BSH, OUT], F32, kind="ExternalOutput")

    def bcast(ap: bass.AP, parts: int) -> bass.AP:
        # replicate a [1, N] DRAM row across `parts` partitions
        return bass.AP(tensor=ap.tensor, offset=ap.offset, ap=[[0, parts]] + ap.ap[1:])

    with ExitStack() as ctx:
        tc = ctx.enter_context(tile.TileContext(nc))
        consts = ctx.enter_context(tc.tile_pool(name="consts", bufs=1))
        npool = ctx.enter_context(tc.tile_pool(name="noise", bufs=NG))
        vpool = ctx.enter_context(tc.tile_pool(name="v", bufs=1))
        ps1 = ctx.enter_context(tc.tile_pool(name="ps1", bufs=4, space="PSUM"))
        ps2 = ctx.enter_context(tc.tile_pool(name="ps2", bufs=1, space="PSUM"))

        # ---- constants ride the ACT HWDGE ring so they land immediately,
        # in parallel with the noise stream on the SP ring. g3 gates every
        # stage-1 matmul, so it must not queue behind 8 MB of noise.
        g3_sb = consts.tile([NS, 3], F16, tag="g3")
        nc.scalar.dma_start(out=g3_sb[:], in_=g3_d[:])
        c_sb = consts.tile([128, 3 * BSH], F32, tag="c")
        nc.scalar.dma_start(out=c_sb[:], in_=bcast(c_d[:], 128))
        bias_sb = consts.tile([BSH, OUT], F32, tag="bias")
        nc.scalar.dma_start(out=bias_sb[:], in_=bcast(bias_d[:], BSH))

        # ---- noise shard (fp16, cast on host), chunked so compute chases ----
        nview = noise_d[:].rearrange("b s i -> s b i")  # [NS, BSH, P]
        noise_t = []
        for g in range(NG):
            t = npool.tile([NS, GB, P], F16, name=f"noise{g}", tag="noise")
            nc.sync.dma_start(out=t[:], in_=nview[:, g * GB : (g + 1) * GB, :])
            noise_t.append(t)

        # mcat queues behind the noise on the SP ring: it is only needed for
        # stage 2, and putting it here keeps it from stealing DMA bandwidth
        # mid noise-stream. Two halves so stage 2 can start on the first 4
        # feature chunks while the rest is still in flight.
        mcat_sb = consts.tile([128, 3 * NQ, OUT], F16, tag="mcat")
        nc.sync.dma_start(out=mcat_sb[:, : 3 * NQ // 2, :],
                          in_=mcat_d[:][:, : 3 * NQ // 2, :])
        nc.sync.dma_start(out=mcat_sb[:, 3 * NQ // 2 :, :],
                          in_=mcat_d[:][:, 3 * NQ // 2 :, :])

        # ---- stage 1: time-collapse matmuls -> psum[i_chunk, (b,{A,L,F})] ----
        ps1_t = [ps1.tile([128, 2 * BSH * 3], F32, name=f"ps1_{i}", tag="ps1")
                 for i in range(4)]
        for g in range(NG):
            for bl in range(GB):
                b = g * GB + bl
                for q in range(NQ):
                    m = _chunk_m(q)
                    co = (q % 2) * (BSH * 3) + b * 3
                    nc.tensor.matmul(
                        ps1_t[q // 2][0:m, co : co + 3],
                        lhsT=noise_t[g][:, bl, q * 128 : q * 128 + m],
                        rhs=g3_sb[:],
                        start=True,
                        stop=True,
                    )

        # ---- psum -> V tiles (fp16): reorder (b,v) -> (v,b) and fold the
        # per-sample feedback scale c_b in (c_sb columns follow V layout) ----
        v_t = [vpool.tile([128, 3 * BSH], F16, name=f"v{q}", tag=f"v{q}")
               for q in range(NQ)]
        nc.vector.memset(v_t[NQ - 1][:], 0.0)  # zero-pad rows 104..127 of last chunk
        for q in range(NQ):
            m = _chunk_m(q)
            src = ps1_t[q // 2][0:m, (q % 2) * (BSH * 3) : (q % 2 + 1) * (BSH * 3)]
            src = src.rearrange("p (b v) -> p v b", v=3)
            dst = v_t[q][0:m, :].rearrange("p (v b) -> p v b", v=3)
            csrc = c_sb[0:m, :].rearrange("p (v b) -> p v b", v=3)
            nc.vector.tensor_mul(dst, src, csrc)

        # ---- stage 2: out[b, j] accumulation over 24 (chunk, variant) tiles ----
        ps_out = ps2.tile([BSH, OUT], F32, tag="ps2")
        idx = 0
        for q in range(NQ):
            for v in range(3):
                t = q * 3 + v
                nc.tensor.matmul(
                    ps_out[:],
                    lhsT=v_t[q][:, v * BSH : (v + 1) * BSH],
                    rhs=mcat_sb[:, t, :],
                    start=(idx == 0),
                    stop=(idx == 3 * NQ - 1),
                )
                idx += 1

        # ---- add bias (c_b already folded into V), store ----
        out_sb = consts.tile([BSH, OUT], F32, tag="outsb")
        nc.vector.tensor_add(out_sb[:], ps_out[:], bias_sb[:])
        nc.sync.dma_start(out=out_d[:], in_=out_sb[:])

    _split_sync_waits(nc)
    _CACHE["nc"] = nc
    return nc


def _host_precompute(decay_param, conv_w, conv_b, out_w, out_b):
    dp = float(np.asarray(decay_param).reshape(-1)[0])
    decay = 0.5 / (1.0 + np.exp(-dp))
    dt = 1.0 / NS
    retain = 1.0 - decay * dt

    s = np.arange(NS, dtype=np.float64)
    gA = (1.0 - retain ** (NS - s)) / (1.0 - retain)
    gL = retain ** (NS - 1 - s)
    g3 = np.zeros((NS, 3), np.float64)
    g3[:, 0] = gA
    g3[:, 1] = gL
    g3[0, 2] = 1.0
    g3 = np.ascontiguousarray(g3.astype(F16_NP))

    conv_w = np.asarray(conv_w, np.float32)
    out_w = np.asarray(out_w, np.float32)
    w_sum = conv_w.sum(axis=2)
    t_sum = out_w @ w_sum              # [OUT, P]
    t0 = out_w @ conv_w[:, :, 0]
    t2 = out_w @ conv_w[:, :, 2]
    r = np.stack([t_sum, -t0, -t2])    # [3, OUT, P]
    r_pad = np.zeros((3, OUT, NQ * 128), np.float32)
    r_pad[:, :, :P] = r
    # mcat[p, q*3+v, j] = r[v, j, q*128+p]  (q-major: halves split cleanly)
    mcat = r_pad.reshape(3, OUT, NQ, 128).transpose(3, 2, 0, 1)  # [128, NQ, 3, OUT]
    mcat = np.ascontiguousarray(mcat.reshape(128, 3 * NQ, OUT).astype(F16_NP))

    bias_vec = (
        out_w @ np.asarray(conv_b, np.float32)
        + np.asarray(out_b, np.float32).reshape(OUT)
    )
    return g3, mcat, bias_vec


def kernel(x, noise, fb_w, fb_b, decay_param, conv_w, conv_b, out_w, out_b,
           _trace=False):
    global LAST_RUN

    x = np.asarray(x, np.float32)
    noise = np.ascontiguousarray(np.asarray(noise, np.float32).astype(F16_NP))

    g3, mcat, bias_vec = _host_precompute(decay_param, conv_w, conv_b, out_w, out_b)
    bias_vec = np.ascontiguousarray(bias_vec.reshape(1, OUT))

    # per-sample feedback scale: sigmoid(x . fb_w + fb_b) * sqrt(dt)/NS
    fb_w = np.asarray(fb_w, np.float32).reshape(IN)
    fb_b = float(np.asarray(fb_b, np.float32).reshape(-1)[0])
    z = x @ fb_w + fb_b
    cvec = (1.0 / (1.0 + np.exp(-z, dtype=np.float64))) * (np.sqrt(1.0 / NS) / NS)
    cvec = cvec.reshape(B).astype(np.float32)

    nc = _build_program()

    in_maps = []
    for c in range(NCORES):
        sl = slice(c * BSH, (c + 1) * BSH)
        in_maps.append(
            {
                "noise_sh": noise[sl],
                "g3": g3,
                "mcat": mcat,
                "cvec": np.ascontiguousarray(
                    np.tile(cvec[sl], 3).reshape(1, 3 * BSH)),
                "biasv": bias_vec,
            }
        )

    res = run_bass_kernel_spmd(nc, in_maps, core_ids=list(range(NCORES)),
                               trace=_trace)
    LAST_RUN = res
    out = np.concatenate([m["out"] for m in res.results], axis=0)
    return out.astype(np.float32)


# revision 13
# speedup vs baseline: 1.0354x; 1.0354x over previous
"""Trainium2 Bass kernel for nn_BICEPNeuralLayer.

Math: the reference module (Euler-Maruyama SDE scan -> Conv1d over time ->
time-mean -> linear projection) is LINEAR in the noise tensor, so the whole
pipeline collapses algebraically:

  paths[t] = c_b * sum_s retain^(t-s) eps_s          (c_b = feedback_b*sqrt(dt))
  mean_t(conv(paths)) folds to per-timestep weights on eps:
     out[b] = (c_b/NS) * (Tsum @ A[b] - T0 @ L[b] - T2 @ F[b]) + bias
  A[b,i] = sum_s gA[s] noise[b,s,i],   gA[s] = (1-retain^(NS-s))/(1-retain)
  L[b,i] = sum_s retain^(NS-1-s) noise[b,s,i]
  F[b,i] = noise[b,0,i]
  Tsum = out_w @ (W0+W1+W2), T0 = out_w @ W0, T2 = out_w @ W2  (Wk = conv_w[:,:,k])
  bias  = out_w @ conv_b + out_b

Device work per core (pure data parallel over batch, 32 samples/core):
  stage 1: per (sample, feature-chunk): matmul(lhsT=noise_chunk[128s x 128i]
           fp16, rhs=G3[128s x 3] fp16) -> psum[i, {A,L,F}] fp32.
           noise is cast fp32->fp16 on the host before upload, halving the
           HBM read (the roofline term) and avoiding the fp32 double-pass
           weight-load penalty on the PE.
  stage 2: 24 accumulating matmuls lhsT=V[128i x 32b] fp16, rhs=Mcat
           [128i x 512j] fp16 -> psum[32b, 512j] fp32, then scale by
           per-sample c_b (host-precomputed sigmoid, tiny) and add bias.
"""

import sys

if "/opt/trn_rl_repo" not in sys.path:
    sys.path.insert(0, "/opt/trn_rl_repo")

from contextlib import ExitStack

import numpy as np

import concourse.bass as bass
import concourse.tile as tile
from concourse import mybir
from concourse.bass_utils import run_bass_kernel_spmd

B, IN, OUT, P, NS = 256, 1024, 512, 1000, 128
NCORES = 8
BSH = B // NCORES  # 32 samples per core
NG = 8             # noise DMA groups per core
GB = BSH // NG     # samples per DMA group (~0.5 MB fp16 per dma_start)
NQ = 8             # feature chunks: 7*128 + 104 = 1000
LASTM = P - (NQ - 1) * 128  # 104

F32 = mybir.dt.float32
F16 = mybir.dt.float16
F16_NP = mybir.dt.np(F16)

_CACHE = {}

LAST_RUN = None  # BassKernelResults of the most recent execution (for test.py)


def _chunk_m(q: int) -> int:
    return 128 if q < NQ - 1 else LASTM


def _split_sync_waits(nc: bass.Bass, max_waits: int = 1) -> int:
    """Walrus in this container accepts at most one sync-wait command per
    instruction. Tile emits instructions (notably the epilogue Drain and any
    op depending on two DMA queues) with several waits. Split the surplus
    onto single-wait NoOps inserted just before, on the same engine, which
    is semantically identical for sem-ge waits."""
    nid = 0
    for fn in nc.m.functions:
        for bb in fn.blocks:
            insts = list(bb.instructions)
            out, changed = [], False
            for inst in insts:
                si = inst.sync_info
                if si is not None and si.on_wait and len(si.on_wait) > max_waits:
                    waits = list(si.on_wait)
                    extra, keep = waits[:-max_waits], waits[-max_waits:]
                    for w in extra:
                        nid += 1
                        out.append(
                            mybir.InstNoOp(
                                name=f"waitsplit-{nid}",
                                sync_info=mybir.SyncInfo(on_wait=[w], on_update=[]),
                                bass_nofuse=True,
                                engine=inst.engine,
                            )
                        )
                    inst.sync_info = mybir.SyncInfo(
                        on_wait=keep, on_update=list(si.on_update)
                    )
                    changed = True
                out.append(inst)
            if changed:
                bb.instructions = out
    return nid


def _build_program() -> bass.Bass:
    if "nc" in _CACHE:
        return _CACHE["nc"]

    nc = bass.Bass()

    noise_d = nc.dram_tensor("noise_sh", [BSH, NS, P], F16, kind="ExternalInput")
    g3_d = nc.dram_tensor("g3", [NS, 3], F16, kind="ExternalInput")
    mcat_d = nc.dram_tensor("mcat", [128, 3 * NQ, OUT], F16, kind="ExternalInput")
    c_d = nc.dram_tensor("cvec", [1, 3 * BSH], F32, kind="ExternalInput")
    bias_d = nc.dram_tensor("biasv", [1, OUT], F32, kind="ExternalInput")
    out_d = nc.dram_tensor("out", [BSH, OUT], F32, kind="ExternalOutput")

    def bcast(ap: bass.AP, parts: int) -> bass.AP:
        # replicate a [1, N] DRAM row across `parts` partitions
        return bass.AP(tensor=ap.tensor, offset=ap.offset, ap=[[0, parts]] + ap.ap[1:])

    with ExitStack() as ctx:
        tc = ctx.enter_context(tile.TileContext(nc))
        consts = ctx.enter_context(tc.tile_pool(name="consts", bufs=1))
        npool = ctx.enter_context(tc.tile_pool(name="noise", bufs=NG))
        vpool = ctx.enter_context(tc.tile_pool(name="v", bufs=1))
        ps1 = ctx.enter_context(tc.tile_pool(name="ps1", bufs=4, space="PSUM"))
        ps2 = ctx.enter_context(tc.tile_pool(name="ps2", bufs=1, space="PSUM"))

        # ---- constants ride the ACT HWDGE ring so they land immediately,
        # in parallel with the noise stream on the SP ring. g3 gates every
        # stage-1 matmul, so it must not queue behind 8 MB of noise.
        g3_sb = consts.tile([NS, 3], F16, tag="g3")
        nc.scalar.dma_start(out=g3_sb[:], in_=g3_d[:])
        c_sb = consts.tile([128, 3 * BSH], F32, tag="c")
        nc.scalar.dma_start(out=c_sb[:], in_=bcast(c_d[:], 128))
        bias_sb = consts.tile([BSH, OUT], F32, tag="bias")
        nc.scalar.dma_start(out=bias_sb[:], in_=bcast(bias_d[:], BSH))

        # ---- noise shard (fp16, cast on host), chunked so compute chases ----
        nview = noise_d[:].rearrange("b s i -> s b i")  # [NS, BSH, P]
        noise_t = []
        for g in range(NG):
            t = npool.tile([NS, GB, P], F16, name=f"noise{g}", tag="noise")
            nc.sync.dma_start(out=t[:], in_=nview[:, g * GB : (g + 1) * GB, :])
            noise_t.append(t)

        # mcat queues behind the noise on the SP ring: it is only needed for
        # stage 2, and putting it here keeps it from stealing DMA bandwidth
        # mid noise-stream. Two halves so stage 2 can start on the first 4
        # feature chunks while the rest is still in flight.
        mcat_sb = consts.tile([128, 3 * NQ, OUT], F16, tag="mcat")
        nc.sync.dma_start(out=mcat_sb[:, : 3 * NQ // 2, :],
                          in_=mcat_d[:][:, : 3 * NQ // 2, :])
        nc.sync.dma_start(out=mcat_sb[:, 3 * NQ // 2 :, :],
                          in_=mcat_d[:][:, 3 * NQ // 2 :, :])

        # ---- stage 1: time-collapse matmuls -> psum[i_chunk, (b,{A,L,F})] ----
        ps1_t = [ps1.tile([128, 2 * BSH * 3], F32, name=f"ps1_{i}", tag="ps1")
                 for i in range(4)]
        for g in range(NG):
            for bl in range(GB):
                b = g * GB + bl
                for q in range(NQ):
                    m = _chunk_m(q)
                    co = (q % 2) * (BSH * 3) + b * 3
                    nc.tensor.matmul(
                        ps1_t[q // 2][0:m, co : co + 3],
                        lhsT=noise_t[g][:, bl, q * 128 : q * 128 + m],
                        rhs=g3_sb[:],
                        start=True,
                        stop=True,
                    )

        # ---- psum -> V tiles (fp16): reorder (b,v) -> (v,b) and fold the
        # per-sample feedback scale c_b in (c_sb columns follow V layout) ----
        v_t = [vpool.tile([128, 3 * BSH], F16, name=f"v{q}", tag=f"v{q}")
               for q in range(NQ)]
        nc.vector.memset(v_t[NQ - 1][:], 0.0)  # zero-pad rows 104..127 of last chunk
        for q in range(NQ):
            m = _chunk_m(q)
            src = ps1_t[q // 2][0:m, (q % 2) * (BSH * 3) : (q % 2 + 1) * (BSH * 3)]
            src = src.rearrange("p (b v) -> p v b", v=3)
            dst = v_t[q][0:m, :].rearrange("p (v b) -> p v b", v=3)
            csrc = c_sb[0:m, :].rearrange("p (v b) -> p v b", v=3)
            nc.vector.tensor_mul(dst, src, csrc)

        # ---- stage 2: out[b, j] accumulation over 24 (chunk, variant) tiles ----
        ps_out = ps2.tile([BSH, OUT], F32, tag="ps2")
        idx = 0
        for q in range(NQ):
            for v in range(3):
                t = q * 3 + v
                nc.tensor.matmul(
                    ps_out[:],
                    lhsT=v_t[q][:, v * BSH : (v + 1) * BSH],
                    rhs=mcat_sb[:, t, :],
                    start=(idx == 0),
                    stop=(idx == 3 * NQ - 1),
                )
                idx += 1

        # ---- add bias (c_b already folded into V), store ----
        out_sb = consts.tile([BSH, OUT], F32, tag="outsb")
        nc.vector.tensor_add(out_sb[:], ps_out[:], bias_sb[:])
        nc.sync.dma_start(out=out_d[:], in_=out_sb[:])

    _split_sync_waits(nc)
    _CACHE["nc"] = nc
    return nc


def _host_precompute(decay_param, conv_w, conv_b, out_w, out_b):
    dp = float(np.asarray(decay_param).reshape(-1)[0])
    decay = 0.5 / (1.0 + np.exp(-dp))
    dt = 1.0 / NS
    retain = 1.0 - decay * dt

    s = np.arange(NS, dtype=np.float64)
    gA = (1.0 - retain ** (NS - s)) / (1.0 - retain)
    gL = retain ** (NS - 1 - s)
    g3 = np.zeros((NS, 3), np.float64)
    g3[:, 0] = gA
    g3[:, 1] = gL
    g3[0, 2] = 1.0
    g3 = np.ascontiguousarray(g3.astype(F16_NP))

    conv_w = np.asarray(conv_w, np.float32)
    out_w = np.asarray(out_w, np.float32)
    w_sum = conv_w.sum(axis=2)
    t_sum = out_w @ w_sum              # [OUT, P]
    t0 = out_w @ conv_w[:, :, 0]
    t2 = out_w @ conv_w[:, :, 2]
    r = np.stack([t_sum, -t0, -t2])    # [3, OUT, P]
    r_pad = np.zeros((3, OUT, NQ * 128), np.float32)
    r_pad[:, :, :P] = r
    # mcat[p, q*3+v, j] = r[v, j, q*128+p]  (q-major: halves split cleanly)
    mcat = r_pad.reshape(3, OUT, NQ, 128).transpose(3, 2, 0, 1)  # [128, NQ, 3, OUT]
    mcat = np.ascontiguousarray(mcat.reshape(128, 3 * NQ, OUT).astype(F16_NP))

    bias_vec = (
        out_w @ np.asarray(conv_b, np.float32)
        + np.asarray(out_b, np.float32).reshape(OUT)
    )
    return g3, mcat, bias_vec


def kernel(x, noise, fb_w, fb_b, decay_param, conv_w, conv_b, out_w, out_b,
           _trace=False):
    global LAST_RUN

    x = np.asarray(x, np.float32)
    noise = np.ascontiguousarray(np.asarray(noise, np.float32).astype(F16_NP))

    g3, mcat, bias_vec = _host_precompute(decay_param, conv_w, conv_b, out_w, out_b)
    bias_vec = np.ascontiguousarray(bias_vec.reshape(1, OUT))

    # per-sample feedback scale: sigmoid(x . fb_w + fb_b) * sqrt(dt)/NS
    fb_w = np.asarray(fb_w, np.float32).reshape(IN)
    fb_b = float(np.asarray(fb_b, np.float32).reshape(-1)[0])
    z = x @ fb_w + fb_b
    cvec = (1.0 / (1.0 + np.exp(-z, dtype=np.float64))) * (np.sqrt(1.0 / NS) / NS)
    cvec = cvec.reshape(B).astype(np.float32)

    nc = _build_program()

    in_maps = []
    for c in range(NCORES):
        sl = slice(c * BSH, (c + 1) * BSH)
        in_maps.append(
            {
                "noise_sh": noise[sl],
                "g3": g3,
                "mcat": mcat,
                "cvec": np.ascontiguousarray(
                    np.tile(cvec[sl], 3).reshape(1, 3 * BSH)),
                "biasv": bias_vec,
            }
        )

    res = run_bass_kernel_spmd(nc, in_maps, core_ids=list(range(NCORES)),
                               trace=_trace)
    LAST_RUN = res
    out = np.concatenate([m["out"] for m in res.results], axis=0)
    return out.astype(np.float32)
